# revision 51
# baseline (speedup 1.0000x reference)
"""Causal multi-head attention (B=2, N=2048, C=768, H=12, D=64) on 8 trn2 cores.

Sharding: 8 cores = 2 batches x 4 head-blocks (3 heads each). Each core
computes q/k/v projections for its 3 heads, causal flash-attention, and a
partial output projection (its 192 columns of Wo). Host sums the 4 partials
per batch (the "all-reduce") and adds the bias during the gather.

v6 structure:
- PV matmuls in [query_part, d_free] layout (lhsT = pt chunks, rhs =
  v[128, 65]): 65 PE rows per (kb, qb) instead of 512, causally exact.
  Each (head, qb) region accumulates as one contiguous start->stop run
  (a psum bank cannot interleave two accumulation chains), sequential
  regions share banks safely.
- The softmax Z lands in the free dim, so normalize is a per-partition
  reciprocal-broadcast multiply on DVE; PE transposes pack the normalized
  [q, 3, 64] back to the [hd, q] layout ([h0;h1] partitions + h2) the
  output projection consumes. No partition_broadcast, no gather DMAs.
- Fillers (projection / output-projection units) are GENERATORS yielding
  every ~2 matmuls; one pull after each S-group matches the PE deficit
  during the exp chase without stalling the Activation engine.
- Weights stored partition-major on the host so DMAs are 128-descriptor;
  critical loads ride the SP queue; bulk loads ride gpsimd SWDGE.
- PE p-state warmup via dummy matmuls; per-pair finish; last block runs
  qb-major bursts with half-width output projection and 4-piece stores so
  the tail overlaps the exp chase.
"""

import numpy as np
import ml_dtypes

B, N, C = 2, 2048, 768
H, D = 12, 64
HL = 3            # heads per core
HD = HL * D       # 192
KC = C // 128     # 6 contraction chunks
NT = N // 512     # 4 query blocks
SCALE = D ** -0.5  # 0.125

_CACHED_NC = None


def _build_nc():
    global _CACHED_NC
    if _CACHED_NC is not None:
        return _CACHED_NC
    import concourse.mybir as mybir
    import concourse.tile as tile
    from concourse import bacc

    f32 = mybir.dt.float32
    bf16 = mybir.dt.bfloat16
    Exp = mybir.ActivationFunctionType.Exp
    is_ge = mybir.AluOpType.is_ge
    is_eq = mybir.AluOpType.is_equal

    nc = bacc.Bacc("TRN2", target_bir_lowering=False, debug=False,
                   enable_asserts=False, num_devices=8)
    xt_d = nc.dram_tensor("xt", [C, N], bf16, kind="ExternalInput").ap()
    wqk_d = nc.dram_tensor("wqk", [3, 128, KC, 128], bf16,
                           kind="ExternalInput").ap()
    wv_d = nc.dram_tensor("wv", [128, KC, 192], bf16,
                          kind="ExternalInput").ap()
    wo_d = nc.dram_tensor("wo", [HD, C], bf16, kind="ExternalInput").ap()
    yt_d = nc.dram_tensor("yt", [C, N], bf16, kind="ExternalOutput").ap()

    # per-head (m-chunk, base-partition) in qkt_sb. K2 lives in k2t_sb.
    QLOC = [(0, 0), (0, 64), (2, 0)]
    KLOC = [(1, 0), (1, 64), None]

    with tile.TileContext(nc) as tc:
        with (
            tc.tile_pool(name="wpool", bufs=1) as wpool,
            tc.tile_pool(name="qkpool", bufs=1) as qkpool,
            tc.tile_pool(name="vpool", bufs=1) as vpool,
            tc.tile_pool(name="opool", bufs=2) as opool,
            tc.tile_pool(name="ppsum", bufs=2, space="PSUM") as ppsum,
            tc.tile_pool(name="stpsum", bufs=2, space="PSUM") as stpsum,
            tc.tile_pool(name="pvpsum", bufs=2, space="PSUM") as pvpsum,
        ):
            wqk_sb = wpool.tile([128, KC, 384], bf16)
            wv_sb = wpool.tile([128, KC, 192], bf16)
            wopk_sb = wpool.tile([128, C], bf16)
            wo2_sb = wpool.tile([64, C], bf16)
            qkt_sb = qkpool.tile([128, 3, N], bf16)
            k2t_sb = qkpool.tile([64, N], bf16)
            v_sb = vpool.tile([128, HL * 16, 65], bf16)
            ones_sb = wpool.tile([128, 1], bf16)
            ident_sb = wpool.tile([128, 128], bf16)

            xpool_cm = tc.tile_pool(name="xpool", bufs=1)
            xpool = xpool_cm.__enter__()
            xt_sb = xpool.tile([128, KC, N], bf16)

            # ---- PE p-state warmup (ramp completes before real work) ----
            xt_r = xt_d.rearrange("(c p) n -> p c n", p=128)
            dummy_sb = wpool.tile([128, 512], bf16)
            nc.vector.memset(dummy_sb[:], 1.0)
            wups = ppsum.tile([128, 512], f32, tag="proj")
            for _ in range(7):
                nc.tensor.matmul(wups[:], dummy_sb[:, 0:128], dummy_sb[:],
                                 start=True, stop=True)

            # ---- loads ----
            nc.sync.dma_start(wqk_sb[:, :, 0:128], wqk_d[0])
            nc.gpsimd.dma_start(xt_sb[:, 0:3, 0:512], xt_r[:, 0:3, 0:512])
            nc.sync.dma_start(xt_sb[:, 3:6, 0:512], xt_r[:, 3:6, 0:512])
            nc.gpsimd.dma_start(wv_sb[:], wv_d[:])
            nc.scalar.dma_start(wqk_sb[:, :, 128:256], wqk_d[1])
            nc.scalar.dma_start(wqk_sb[:, :, 256:384], wqk_d[2])
            nc.vector.memset(ones_sb[:], 1.0)
            nc.vector.tensor_copy(v_sb[:, :, 64:65],
                                  ones_sb[:].to_broadcast([128, HL * 16, 1]))
            nc.gpsimd.memset(ident_sb[:], 1.0)
            nc.gpsimd.affine_select(ident_sb[:], ident_sb[:],
                                    pattern=[[1, 128]], compare_op=is_eq,
                                    fill=0.0, base=0, channel_multiplier=-1)
            nc.scalar.dma_start(xt_sb[:, :, 512:1024], xt_r[:, :, 512:1024])
            nc.scalar.dma_start(xt_sb[:, :, 1024:1536], xt_r[:, :, 1024:1536])
            nc.scalar.dma_start(xt_sb[:, :, 1536:2048], xt_r[:, :, 1536:2048])
            nc.gpsimd.dma_start(wopk_sb[:], wo_d[0:128, :])
            nc.gpsimd.dma_start(wo2_sb[:], wo_d[128:HD, :])

            # ---- filler units as generators (~2 matmuls per pull) ----
            def proj_units(t):
                ts_ = slice(t * 512, (t + 1) * 512)
                units = []

                def qk_unit(m):
                    def emit():
                        ps = ppsum.tile([128, 512], f32, tag="proj")
                        for c in range(KC):
                            nc.tensor.matmul(ps[:], wqk_sb[:, c, m * 128:(m + 1) * 128],
                                             xt_sb[:, c, ts_],
                                             start=(c == 0), stop=(c == KC - 1))
                            if c % 2 == 1 and c < KC - 1:
                                yield 427
                        t0_ = t * 512
                        nc.vector.tensor_copy(qkt_sb[:, m, t0_:t0_ + 256],
                                              ps[:, 0:256])
                        nc.vector.tensor_copy(qkt_sb[:, m, t0_ + 256:t0_ + 512],
                                              ps[:, 256:512])
                        if m == 2:
                            nc.gpsimd.dma_start(k2t_sb[:, ts_],
                                                qkt_sb[64:128, 2, ts_])
                        yield 447
                    return emit

                def v_unit(q):
                    def emit():
                        tok = t * 512 + q * 128
                        i = t * 4 + q
                        ps = ppsum.tile([128, 512], f32, tag="proj")
                        for c in range(KC):
                            nc.tensor.matmul(ps[:, 0:192], xt_sb[:, c, tok:tok + 128],
                                             wv_sb[:, c, :],
                                             start=(c == 0), stop=(c == KC - 1))
                            if c == 2:
                                yield 240
                        vsrc = ps[:, 0:HD].rearrange("p (h d) -> p h d", h=HL)
                        nc.vector.tensor_copy(v_sb[:, i::16, 0:64], vsrc)
                        yield 260
                    return emit

                units.append(qk_unit(0))
                for q in range(4):
                    units.append(v_unit(q))
                units.append(qk_unit(1))
                units.append(qk_unit(2))
                return units

            def run_unit(u):
                for c in u():
                    pe_clock[0] += c

            # ---- global software pipeline ----------------------------
            # One continuous ACT-paced stream of S-groups (all blocks, heads
            # in order); between groups, PE pulls deficit-sized chunks from
            # a FIFO work queue holding bursts, finishes, projections and
            # output projections. FIFO order IS emission order, which
            # guarantees producer-before-consumer program order; lazy tile
            # allocation inside generator bodies keeps pool cycling sound.
            from collections import deque
            bpools_cm = [
                tc.tile_pool(name="ptpool", bufs=40),
                tc.tile_pool(name="smpool", bufs=2),
                tc.tile_pool(name="ytpool", bufs=2),
            ]
            ptpool, smpool, ytpool = [cm.__enter__() for cm in bpools_cm]
            work = deque()
            cur = [None]
            pe_clock = [0.0]   # cumulative PE-busy estimate
            act_clock = [0.0]  # cumulative ACT(exp)-busy estimate

            def pull(budget):
                while budget > 0:
                    if cur[0] is None:
                        if not work:
                            return
                        cur[0] = work.popleft()[1]()
                    try:
                        budget -= next(cur[0])
                    except StopIteration:
                        cur[0] = None

            def force_one():
                if cur[0] is None:
                    if not work:
                        return False
                    cur[0] = work.popleft()[1]()
                try:
                    pe_clock[0] += next(cur[0])
                except StopIteration:
                    cur[0] = None
                return True

            def drain_through(names):
                names = set(names)
                while names & {n for n, _ in work} or cur[0] is not None:
                    if not force_one():
                        return

            def drain_all():
                while work or cur[0] is not None:
                    if not force_one():
                        return

            pts_all = {}   # (j, h) -> list of pt tiles
            odict = {}     # j -> (o_pk, o_j2, zr, o_nrm); ("yt", j) -> yt_sb
            pvt_all = {}   # j -> [pv01, pv23]

            def st_group(j, h, g):
                qs0 = j * 512
                qm, qp = QLOC[h]
                glo = 128 * (2 * g - 4 * j) if 2 * g >= 4 * j else 0
                st = stpsum.tile([128, 2, 512], f32, tag="st")
                pt = ptpool.tile([128, 2, 512], bf16, tag="pt")
                pts_all.setdefault((j, h), []).append(pt)
                rows = 0
                for li in range(2):
                    kb = 2 * g + li
                    gl = 128 * (kb - 4 * j) if kb >= 4 * j else 0
                    rows += 512 - gl
                    if h < 2:
                        km, kp = KLOC[h]
                        lhsk = qkt_sb[kp:kp + 64, km, kb * 128:(kb + 1) * 128]
                    else:
                        lhsk = k2t_sb[:, kb * 128:(kb + 1) * 128]
                    nc.tensor.matmul(
                        st[:, li, gl:512], lhsk,
                        qkt_sb[qp:qp + 64, qm, qs0 + gl:qs0 + 512],
                        start=True, stop=True)
                nc.scalar.activation(pt[:, :, glo:512], st[:, :, glo:512],
                                     Exp, scale=SCALE)
                for li in range(2):
                    kb = 2 * g + li
                    if kb >= 4 * j:  # diagonal: mask the 128-wide band
                        di = kb - 4 * j
                        blk = pt[:, li, 128 * di:128 * (di + 1)]
                        nc.gpsimd.affine_select(
                            blk, blk, pattern=[[1, 128]], compare_op=is_ge,
                            fill=0.0, base=0, channel_multiplier=-1)
                return 2 * (512 - glo) * 0.833 + 185 - rows * 0.4167 - 300

            def finish_pair(j, pair):
                if j not in odict:
                    zr = smpool.tile([128, 2, 2, HL, 1], f32, tag="zr")
                    o_nrm = smpool.tile([128, 4, HL, 64], bf16, tag="on")
                    o_pk = opool.tile([128, 512], bf16, tag="opk")
                    o_j2 = opool.tile([64, 512], bf16, tag="oj2")
                    odict[j] = (o_pk, o_j2, zr, o_nrm)
                o_pk, o_j2, zr, o_nrm = odict[j]
                ps = pvt_all[j][pair]
                tp = ppsum.tile([128, 512], bf16, tag="proj")
                nc.vector.reciprocal_approx_fast(
                    zr[:, pair, :, :, :].rearrange("p a h 1 -> p (a h) 1"),
                    ps[:, :, :, 64:65].rearrange("p a h 1 -> p (a h) 1"))
                nc.vector.tensor_mul(
                    o_nrm[:, pair * 2:pair * 2 + 2, :, :], ps[:, :, :, 0:64],
                    zr[:, pair, :, :, :].to_broadcast([128, 2, HL, 64]))
                for ql in range(2):
                    qb = pair * 2 + ql
                    nc.tensor.transpose(tp[:, ql * 128:(ql + 1) * 128],
                                        o_nrm[:, qb, 0:2, :], ident_sb[:])
                    nc.tensor.transpose(
                        tp[0:64, 256 + ql * 128:256 + (ql + 1) * 128],
                        o_nrm[:, qb, 2, :], ident_sb[:])
                hsl = slice(pair * 256, (pair + 1) * 256)
                nc.vector.tensor_copy(o_pk[:, hsl], tp[:, 0:256])
                nc.vector.tensor_copy(o_j2[:, hsl], tp[0:64, 256:512])
                yield 300

            def bursts_gen(j, h):
                def gen():
                    if j not in pvt_all:
                        pv01 = pvpsum.tile([128, 2, HL, 65], f32, tag="pv")
                        pv23 = pvpsum.tile([128, 2, HL, 65], f32, tag="pv")
                        pvt_all[j] = [pv01, pv23]
                    pvt = pvt_all[j]
                    pts = pts_all[(j, h)]
                    for qb in range(4):
                        last_kb = 4 * j + qb
                        for kb in range(last_kb + 1):
                            nc.tensor.matmul(
                                pvt[qb // 2][:, qb % 2, h, :],
                                pts[kb // 2][:, kb % 2, qb * 128:(qb + 1) * 128],
                                v_sb[:, h * 16 + kb, :],
                                start=(kb == 0), stop=(kb == last_kb))
                        yield (last_kb + 1) * 65 * 0.4167
                        if h == 2 and qb % 2 == 1:
                            yield from finish_pair(j, qb // 2)
                return gen

            def yt_unit(j, ob, last):
                qs = slice(j * 512, (j + 1) * 512)
                yt_r = yt_d.rearrange("(c p) n -> p c n", p=128)

                def gen():
                    o_pk, o_j2 = odict[j][0], odict[j][1]
                    if ob == 0:
                        odict[("yt", j)] = ytpool.tile(
                            [128, KC, 512], bf16, tag="yt", name=f"yt{j}")
                    yt_sb = odict[("yt", j)]
                    ps = ppsum.tile([128, 512], f32, tag="proj")
                    nc.tensor.matmul(ps[:], wopk_sb[:, ob * 128:(ob + 1) * 128],
                                     o_pk[:], start=True, stop=False)
                    nc.tensor.matmul(ps[:], wo2_sb[:, ob * 128:(ob + 1) * 128],
                                     o_j2[:], start=False, stop=True)
                    yield 427
                    if last and ob == KC - 1:
                        # final ob: one ACT copy (DVE pays ~800ns psum sem
                        # latency here), smallest-possible last store
                        nc.scalar.copy(yt_sb[:, ob, :], ps[:])
                        nc.sync.dma_start(yt_r[:, 4:5, qs], yt_sb[:, 4:5, :])
                        nc.sync.dma_start(yt_r[:, 5:6, qs], yt_sb[:, 5:6, :])
                    elif last:
                        if ob % 2 == 0:
                            nc.scalar.copy(yt_sb[:, ob, :], ps[:])
                        else:
                            nc.vector.tensor_copy(yt_sb[:, ob, :], ps[:])
                            cp = slice(ob - 1, ob + 1)
                            nc.sync.dma_start(yt_r[:, cp, qs], yt_sb[:, cp, :])
                    else:
                        nc.vector.tensor_copy(yt_sb[:, ob, :], ps[:])
                        if ob % 2 == 1:  # store each completed c-pair
                            # sync hwdge: pool swdge generation serializes on
                            # the pool engine right when these drain late
                            cp = slice(ob - 1, ob + 1)
                            nc.sync.dma_start(yt_r[:, cp, qs], yt_sb[:, cp, :])
                    yield 20
                return gen

            # ---- prologue + stream ----
            # blocks ascend so each block's projection work for the NEXT
            # block fits its own (growing) exp-chase window. Only qk0/qk1 of
            # t0 are eager: the exp stream starts ~5us earlier; everything
            # else flows through the clock-driven FIFO. Per-head prereqs:
            # qka{t} (m0/m1) gates block t's head 0, qkb{t} (m2+k2t) gates
            # its head 2.
            pu = {t: proj_units(t) for t in range(4)}
            run_unit(pu[0][0])
            run_unit(pu[0][5])

            def enqueue_proj(t, kinds):
                names = {0: "qka", 1: "v", 2: "v", 3: "v", 4: "v",
                         5: "qka", 6: "qkb"}
                # qk ahead of v so a forced qk drain never drags v units
                for kind in ("qka", "qkb", "v"):
                    if kind not in kinds:
                        continue
                    for i, u in enumerate(pu[t]):
                        if names[i] == kind:
                            work.append((kind + str(t), u))

            enqueue_proj(0, ("qkb", "v"))
            # yt(1)/yt(2) enter the queue one block late so block 3's big
            # exp-chase window (deficit ~9us) has enough PE work to pull.
            pending_yt = {j: [] for j in range(4)}
            for j in range(4):
                last = j == 3
                if j < 3:
                    enqueue_proj(j + 1, ("qka", "qkb", "v"))
                for u in pending_yt[j]:
                    work.append(u)
                for h in range(HL):
                    if h == 0 and j > 0:
                        drain_through([f"qka{j}"])
                    elif h == 2:
                        drain_through([f"qkb{j}"])
                    for g in range(2 * (j + 1)):
                        budget = st_group(j, h, g)
                        pull(budget)
                    work.append((f"b{j}", bursts_gen(j, h)))
                units = [(f"yt{j}", yt_unit(j, ob, last)) for ob in range(KC)]
                if j in (0, 1):
                    pending_yt[j + 1] = units
                else:
                    work.extend(units)
            drain_all()

            for cm in reversed(bpools_cm):
                cm.__exit__(None, None, None)
            xpool_cm.__exit__(None, None, None)

    nc.compile()
    _CACHED_NC = nc
    return nc


def _make_in_maps(x, Wq, Wk, Wv, Wo):
    bf16 = ml_dtypes.bfloat16
    x = np.asarray(x, np.float32)
    Wq = np.asarray(Wq, np.float32)
    Wk = np.asarray(Wk, np.float32)
    Wv = np.asarray(Wv, np.float32)
    Wo = np.asarray(Wo, np.float32)
    in_maps = []
    for c in range(8):
        b, hb = divmod(c, 4)
        s = slice(hb * HD, (hb + 1) * HD)
        wq_s = Wq[s].T  # (768, 192)
        wk_s = Wk[s].T
        # m0 = [Q0|Q1], m1 = [K0|K1], m2 = [Q2|K2]; each m-block stored
        # partition-major [128p, 6c, 128m] so every partition's DMA payload
        # is one contiguous 1536B run (128 descriptors, not 768).
        blocks = [np.concatenate([wq_s[:, 0:128], wk_s[:, 0:128]], axis=1),
                  np.concatenate([wq_s[:, 128:HD], wk_s[:, 128:HD]], axis=1)]
        wqk = np.stack([
            blocks[0][:, 0:128], blocks[0][:, 128:256], blocks[1],
        ]).reshape(3, KC, 128, 128).transpose(0, 2, 1, 3)  # (3, p, c, m)
        wv_pm = Wv[s].T.reshape(KC, 128, 192).transpose(1, 0, 2)  # (p, c, m)
        in_maps.append({
            "xt": np.ascontiguousarray(x[b].T).astype(bf16),
            "wqk": np.ascontiguousarray(wqk).astype(bf16),
            "wv": np.ascontiguousarray(wv_pm).astype(bf16),
            "wo": np.ascontiguousarray(Wo[:, s].T).astype(bf16),
        })
    return in_maps


def _gather(results, bo):
    out = np.zeros((B, N, C), np.float32)
    for c in range(8):
        out[c // 4] += results[c]["yt"].astype(np.float32).T
    out += np.asarray(bo, np.float32)[None, None, :]
    return out


def kernel(x, Wq, Wk, Wv, Wo, bo):
    from concourse.bass_utils import run_bass_kernel_spmd
    nc = _build_nc()
    in_maps = _make_in_maps(x, Wq, Wk, Wv, Wo)
    try:
        res = run_bass_kernel_spmd(nc, in_maps, core_ids=list(range(8)))
    except ModuleNotFoundError:
        # BASS_TRACE set but this axon deployment lacks the NTFF hook module
        import os
        os.environ["BASS_NEVER_TRACE"] = "1"
        res = run_bass_kernel_spmd(nc, in_maps, core_ids=list(range(8)))
    return _gather(res.results, bo)


# revision 52
# speedup vs baseline: 1.0016x; 1.0016x over previous
"""Causal multi-head attention (B=2, N=2048, C=768, H=12, D=64) on 8 trn2 cores.

Sharding: 8 cores = 2 batches x 4 head-blocks (3 heads each). Each core
computes q/k/v projections for its 3 heads, causal flash-attention, and a
partial output projection (its 192 columns of Wo). Host sums the 4 partials
per batch (the "all-reduce") and adds the bias during the gather.

v6 structure:
- PV matmuls in [query_part, d_free] layout (lhsT = pt chunks, rhs =
  v[128, 65]): 65 PE rows per (kb, qb) instead of 512, causally exact.
  Each (head, qb) region accumulates as one contiguous start->stop run
  (a psum bank cannot interleave two accumulation chains), sequential
  regions share banks safely.
- The softmax Z lands in the free dim, so normalize is a per-partition
  reciprocal-broadcast multiply on DVE; PE transposes pack the normalized
  [q, 3, 64] back to the [hd, q] layout ([h0;h1] partitions + h2) the
  output projection consumes. No partition_broadcast, no gather DMAs.
- Fillers (projection / output-projection units) are GENERATORS yielding
  every ~2 matmuls; one pull after each S-group matches the PE deficit
  during the exp chase without stalling the Activation engine.
- Weights stored partition-major on the host so DMAs are 128-descriptor;
  critical loads ride the SP queue; bulk loads ride gpsimd SWDGE.
- PE p-state warmup via dummy matmuls; per-pair finish; last block runs
  qb-major bursts with half-width output projection and 4-piece stores so
  the tail overlaps the exp chase.
"""

import numpy as np
import ml_dtypes

B, N, C = 2, 2048, 768
H, D = 12, 64
HL = 3            # heads per core
HD = HL * D       # 192
KC = C // 128     # 6 contraction chunks
NT = N // 512     # 4 query blocks
SCALE = D ** -0.5  # 0.125

_CACHED_NC = None


def _build_nc():
    global _CACHED_NC
    if _CACHED_NC is not None:
        return _CACHED_NC
    import concourse.mybir as mybir
    import concourse.tile as tile
    from concourse import bacc

    f32 = mybir.dt.float32
    bf16 = mybir.dt.bfloat16
    Exp = mybir.ActivationFunctionType.Exp
    is_ge = mybir.AluOpType.is_ge
    is_eq = mybir.AluOpType.is_equal

    nc = bacc.Bacc("TRN2", target_bir_lowering=False, debug=False,
                   enable_asserts=False, num_devices=8)
    xt_d = nc.dram_tensor("xt", [C, N], bf16, kind="ExternalInput").ap()
    wqk_d = nc.dram_tensor("wqk", [3, 128, KC, 128], bf16,
                           kind="ExternalInput").ap()
    wv_d = nc.dram_tensor("wv", [128, KC, 192], bf16,
                          kind="ExternalInput").ap()
    wo_d = nc.dram_tensor("wo", [HD, C], bf16, kind="ExternalInput").ap()
    yt_d = nc.dram_tensor("yt", [C, N], bf16, kind="ExternalOutput").ap()

    # per-head (m-chunk, base-partition) in qkt_sb. K2 lives in k2t_sb.
    QLOC = [(0, 0), (0, 64), (2, 0)]
    KLOC = [(1, 0), (1, 64), None]

    with tile.TileContext(nc) as tc:
        with (
            tc.tile_pool(name="wpool", bufs=1) as wpool,
            tc.tile_pool(name="qkpool", bufs=1) as qkpool,
            tc.tile_pool(name="vpool", bufs=1) as vpool,
            tc.tile_pool(name="opool", bufs=2) as opool,
            tc.tile_pool(name="ppsum", bufs=2, space="PSUM") as ppsum,
            tc.tile_pool(name="stpsum", bufs=2, space="PSUM") as stpsum,
            tc.tile_pool(name="pvpsum", bufs=2, space="PSUM") as pvpsum,
        ):
            wqk_sb = wpool.tile([128, KC, 384], bf16)
            wv_sb = wpool.tile([128, KC, 192], bf16)
            wopk_sb = wpool.tile([128, C], bf16)
            wo2_sb = wpool.tile([64, C], bf16)
            qkt_sb = qkpool.tile([128, 3, N], bf16)
            k2t_sb = qkpool.tile([64, N], bf16)
            v_sb = vpool.tile([128, HL * 16, 65], bf16)
            ones_sb = wpool.tile([128, 1], bf16)
            ident_sb = wpool.tile([128, 128], bf16)

            xpool_cm = tc.tile_pool(name="xpool", bufs=1)
            xpool = xpool_cm.__enter__()
            xt_sb = xpool.tile([128, KC, N], bf16)

            # ---- PE p-state warmup (ramp completes before real work) ----
            xt_r = xt_d.rearrange("(c p) n -> p c n", p=128)
            dummy_sb = wpool.tile([128, 512], bf16)
            nc.vector.memset(dummy_sb[:], 1.0)
            wups = ppsum.tile([128, 512], f32, tag="proj")
            for _ in range(9):
                nc.tensor.matmul(wups[:], dummy_sb[:, 0:128], dummy_sb[:],
                                 start=True, stop=True)

            # ---- loads ----
            nc.sync.dma_start(wqk_sb[:, :, 0:128], wqk_d[0])
            nc.gpsimd.dma_start(xt_sb[:, 0:3, 0:512], xt_r[:, 0:3, 0:512])
            nc.sync.dma_start(xt_sb[:, 3:6, 0:512], xt_r[:, 3:6, 0:512])
            nc.gpsimd.dma_start(wv_sb[:], wv_d[:])
            nc.scalar.dma_start(wqk_sb[:, :, 128:256], wqk_d[1])
            nc.scalar.dma_start(wqk_sb[:, :, 256:384], wqk_d[2])
            nc.vector.memset(ones_sb[:], 1.0)
            nc.vector.tensor_copy(v_sb[:, :, 64:65],
                                  ones_sb[:].to_broadcast([128, HL * 16, 1]))
            nc.gpsimd.memset(ident_sb[:], 1.0)
            nc.gpsimd.affine_select(ident_sb[:], ident_sb[:],
                                    pattern=[[1, 128]], compare_op=is_eq,
                                    fill=0.0, base=0, channel_multiplier=-1)
            nc.scalar.dma_start(xt_sb[:, :, 512:1024], xt_r[:, :, 512:1024])
            nc.scalar.dma_start(xt_sb[:, :, 1024:1536], xt_r[:, :, 1024:1536])
            nc.scalar.dma_start(xt_sb[:, :, 1536:2048], xt_r[:, :, 1536:2048])
            nc.gpsimd.dma_start(wopk_sb[:], wo_d[0:128, :])
            nc.gpsimd.dma_start(wo2_sb[:], wo_d[128:HD, :])

            # ---- filler units as generators (~2 matmuls per pull) ----
            def proj_units(t):
                ts_ = slice(t * 512, (t + 1) * 512)
                units = []

                def qk_unit(m):
                    def emit():
                        ps = ppsum.tile([128, 512], f32, tag="proj")
                        for c in range(KC):
                            nc.tensor.matmul(ps[:], wqk_sb[:, c, m * 128:(m + 1) * 128],
                                             xt_sb[:, c, ts_],
                                             start=(c == 0), stop=(c == KC - 1))
                            if c % 2 == 1 and c < KC - 1:
                                yield 427
                        t0_ = t * 512
                        nc.vector.tensor_copy(qkt_sb[:, m, t0_:t0_ + 256],
                                              ps[:, 0:256])
                        nc.vector.tensor_copy(qkt_sb[:, m, t0_ + 256:t0_ + 512],
                                              ps[:, 256:512])
                        if m == 2:
                            nc.gpsimd.dma_start(k2t_sb[:, ts_],
                                                qkt_sb[64:128, 2, ts_])
                        yield 447
                    return emit

                def v_unit(q):
                    def emit():
                        tok = t * 512 + q * 128
                        i = t * 4 + q
                        ps = ppsum.tile([128, 512], f32, tag="proj")
                        for c in range(KC):
                            nc.tensor.matmul(ps[:, 0:192], xt_sb[:, c, tok:tok + 128],
                                             wv_sb[:, c, :],
                                             start=(c == 0), stop=(c == KC - 1))
                            if c == 2:
                                yield 240
                        vsrc = ps[:, 0:HD].rearrange("p (h d) -> p h d", h=HL)
                        nc.vector.tensor_copy(v_sb[:, i::16, 0:64], vsrc)
                        yield 260
                    return emit

                units.append(qk_unit(0))
                for q in range(4):
                    units.append(v_unit(q))
                units.append(qk_unit(1))
                units.append(qk_unit(2))
                return units

            def run_unit(u):
                for c in u():
                    pe_clock[0] += c

            # ---- global software pipeline ----------------------------
            # One continuous ACT-paced stream of S-groups (all blocks, heads
            # in order); between groups, PE pulls deficit-sized chunks from
            # a FIFO work queue holding bursts, finishes, projections and
            # output projections. FIFO order IS emission order, which
            # guarantees producer-before-consumer program order; lazy tile
            # allocation inside generator bodies keeps pool cycling sound.
            from collections import deque
            bpools_cm = [
                tc.tile_pool(name="ptpool", bufs=40),
                tc.tile_pool(name="smpool", bufs=2),
                tc.tile_pool(name="ytpool", bufs=2),
            ]
            ptpool, smpool, ytpool = [cm.__enter__() for cm in bpools_cm]
            work = deque()
            cur = [None]
            pe_clock = [0.0]   # cumulative PE-busy estimate
            act_clock = [0.0]  # cumulative ACT(exp)-busy estimate

            def pull(budget):
                while budget > 0:
                    if cur[0] is None:
                        if not work:
                            return
                        cur[0] = work.popleft()[1]()
                    try:
                        budget -= next(cur[0])
                    except StopIteration:
                        cur[0] = None

            def force_one():
                if cur[0] is None:
                    if not work:
                        return False
                    cur[0] = work.popleft()[1]()
                try:
                    pe_clock[0] += next(cur[0])
                except StopIteration:
                    cur[0] = None
                return True

            def drain_through(names):
                names = set(names)
                while names & {n for n, _ in work} or cur[0] is not None:
                    if not force_one():
                        return

            def drain_all():
                while work or cur[0] is not None:
                    if not force_one():
                        return

            pts_all = {}   # (j, h) -> list of pt tiles
            odict = {}     # j -> (o_pk, o_j2, zr, o_nrm); ("yt", j) -> yt_sb
            pvt_all = {}   # j -> [pv01, pv23]

            def st_group(j, h, g):
                qs0 = j * 512
                qm, qp = QLOC[h]
                glo = 128 * (2 * g - 4 * j) if 2 * g >= 4 * j else 0
                st = stpsum.tile([128, 2, 512], f32, tag="st")
                pt = ptpool.tile([128, 2, 512], bf16, tag="pt")
                pts_all.setdefault((j, h), []).append(pt)
                rows = 0
                for li in range(2):
                    kb = 2 * g + li
                    gl = 128 * (kb - 4 * j) if kb >= 4 * j else 0
                    rows += 512 - gl
                    if h < 2:
                        km, kp = KLOC[h]
                        lhsk = qkt_sb[kp:kp + 64, km, kb * 128:(kb + 1) * 128]
                    else:
                        lhsk = k2t_sb[:, kb * 128:(kb + 1) * 128]
                    nc.tensor.matmul(
                        st[:, li, gl:512], lhsk,
                        qkt_sb[qp:qp + 64, qm, qs0 + gl:qs0 + 512],
                        start=True, stop=True)
                nc.scalar.activation(pt[:, :, glo:512], st[:, :, glo:512],
                                     Exp, scale=SCALE)
                for li in range(2):
                    kb = 2 * g + li
                    if kb >= 4 * j:  # diagonal: mask the 128-wide band
                        di = kb - 4 * j
                        blk = pt[:, li, 128 * di:128 * (di + 1)]
                        nc.gpsimd.affine_select(
                            blk, blk, pattern=[[1, 128]], compare_op=is_ge,
                            fill=0.0, base=0, channel_multiplier=-1)
                return 2 * (512 - glo) * 0.833 + 185 - rows * 0.4167 - 300

            def finish_pair(j, pair):
                if j not in odict:
                    zr = smpool.tile([128, 2, 2, HL, 1], f32, tag="zr")
                    o_nrm = smpool.tile([128, 4, HL, 64], bf16, tag="on")
                    o_pk = opool.tile([128, 512], bf16, tag="opk")
                    o_j2 = opool.tile([64, 512], bf16, tag="oj2")
                    odict[j] = (o_pk, o_j2, zr, o_nrm)
                o_pk, o_j2, zr, o_nrm = odict[j]
                ps = pvt_all[j][pair]
                tp = ppsum.tile([128, 512], bf16, tag="proj")
                nc.vector.reciprocal_approx_fast(
                    zr[:, pair, :, :, :].rearrange("p a h 1 -> p (a h) 1"),
                    ps[:, :, :, 64:65].rearrange("p a h 1 -> p (a h) 1"))
                nc.vector.tensor_mul(
                    o_nrm[:, pair * 2:pair * 2 + 2, :, :], ps[:, :, :, 0:64],
                    zr[:, pair, :, :, :].to_broadcast([128, 2, HL, 64]))
                for ql in range(2):
                    qb = pair * 2 + ql
                    nc.tensor.transpose(tp[:, ql * 128:(ql + 1) * 128],
                                        o_nrm[:, qb, 0:2, :], ident_sb[:])
                    nc.tensor.transpose(
                        tp[0:64, 256 + ql * 128:256 + (ql + 1) * 128],
                        o_nrm[:, qb, 2, :], ident_sb[:])
                hsl = slice(pair * 256, (pair + 1) * 256)
                nc.vector.tensor_copy(o_pk[:, hsl], tp[:, 0:256])
                nc.vector.tensor_copy(o_j2[:, hsl], tp[0:64, 256:512])
                yield 300

            def bursts_gen(j, h):
                def gen():
                    if j not in pvt_all:
                        pv01 = pvpsum.tile([128, 2, HL, 65], f32, tag="pv")
                        pv23 = pvpsum.tile([128, 2, HL, 65], f32, tag="pv")
                        pvt_all[j] = [pv01, pv23]
                    pvt = pvt_all[j]
                    pts = pts_all[(j, h)]
                    for qb in range(4):
                        last_kb = 4 * j + qb
                        for kb in range(last_kb + 1):
                            nc.tensor.matmul(
                                pvt[qb // 2][:, qb % 2, h, :],
                                pts[kb // 2][:, kb % 2, qb * 128:(qb + 1) * 128],
                                v_sb[:, h * 16 + kb, :],
                                start=(kb == 0), stop=(kb == last_kb))
                        yield (last_kb + 1) * 65 * 0.4167
                        if h == 2 and qb % 2 == 1:
                            yield from finish_pair(j, qb // 2)
                return gen

            def yt_unit(j, ob, last):
                qs = slice(j * 512, (j + 1) * 512)
                yt_r = yt_d.rearrange("(c p) n -> p c n", p=128)

                def gen():
                    o_pk, o_j2 = odict[j][0], odict[j][1]
                    if ob == 0:
                        odict[("yt", j)] = ytpool.tile(
                            [128, KC, 512], bf16, tag="yt", name=f"yt{j}")
                    yt_sb = odict[("yt", j)]
                    ps = ppsum.tile([128, 512], f32, tag="proj")
                    nc.tensor.matmul(ps[:], wopk_sb[:, ob * 128:(ob + 1) * 128],
                                     o_pk[:], start=True, stop=False)
                    nc.tensor.matmul(ps[:], wo2_sb[:, ob * 128:(ob + 1) * 128],
                                     o_j2[:], start=False, stop=True)
                    yield 427
                    if last and ob == KC - 1:
                        # final ob: one ACT copy (DVE pays ~800ns psum sem
                        # latency here), smallest-possible last store
                        nc.scalar.copy(yt_sb[:, ob, :], ps[:])
                        nc.sync.dma_start(yt_r[:, 4:5, qs], yt_sb[:, 4:5, :])
                        nc.sync.dma_start(yt_r[:, 5:6, qs], yt_sb[:, 5:6, :])
                    elif last:
                        if ob % 2 == 0:
                            nc.scalar.copy(yt_sb[:, ob, :], ps[:])
                        else:
                            nc.vector.tensor_copy(yt_sb[:, ob, :], ps[:])
                            cp = slice(ob - 1, ob + 1)
                            nc.sync.dma_start(yt_r[:, cp, qs], yt_sb[:, cp, :])
                    else:
                        nc.vector.tensor_copy(yt_sb[:, ob, :], ps[:])
                        if ob % 2 == 1:  # store each completed c-pair
                            # sync hwdge: pool swdge generation serializes on
                            # the pool engine right when these drain late
                            cp = slice(ob - 1, ob + 1)
                            nc.sync.dma_start(yt_r[:, cp, qs], yt_sb[:, cp, :])
                    yield 20
                return gen

            # ---- prologue + stream ----
            # blocks ascend so each block's projection work for the NEXT
            # block fits its own (growing) exp-chase window. Only qk0/qk1 of
            # t0 are eager: the exp stream starts ~5us earlier; everything
            # else flows through the clock-driven FIFO. Per-head prereqs:
            # qka{t} (m0/m1) gates block t's head 0, qkb{t} (m2+k2t) gates
            # its head 2.
            pu = {t: proj_units(t) for t in range(4)}
            run_unit(pu[0][0])
            run_unit(pu[0][5])

            def enqueue_proj(t, kinds):
                names = {0: "qka", 1: "v", 2: "v", 3: "v", 4: "v",
                         5: "qka", 6: "qkb"}
                # qk ahead of v so a forced qk drain never drags v units
                for kind in ("qka", "qkb", "v"):
                    if kind not in kinds:
                        continue
                    for i, u in enumerate(pu[t]):
                        if names[i] == kind:
                            work.append((kind + str(t), u))

            enqueue_proj(0, ("qkb", "v"))
            # yt(1)/yt(2) enter the queue one block late so block 3's big
            # exp-chase window (deficit ~9us) has enough PE work to pull.
            pending_yt = {j: [] for j in range(4)}
            for j in range(4):
                last = j == 3
                if j < 3:
                    enqueue_proj(j + 1, ("qka", "qkb", "v"))
                for u in pending_yt[j]:
                    work.append(u)
                for h in range(HL):
                    if h == 0 and j > 0:
                        drain_through([f"qka{j}"])
                    elif h == 2:
                        drain_through([f"qkb{j}"])
                    for g in range(2 * (j + 1)):
                        budget = st_group(j, h, g)
                        pull(budget)
                    work.append((f"b{j}", bursts_gen(j, h)))
                units = [(f"yt{j}", yt_unit(j, ob, last)) for ob in range(KC)]
                if j in (0, 1):
                    pending_yt[j + 1] = units
                else:
                    work.extend(units)
            drain_all()

            for cm in reversed(bpools_cm):
                cm.__exit__(None, None, None)
            xpool_cm.__exit__(None, None, None)

    nc.compile()
    _CACHED_NC = nc
    return nc


def _make_in_maps(x, Wq, Wk, Wv, Wo):
    bf16 = ml_dtypes.bfloat16
    x = np.asarray(x, np.float32)
    Wq = np.asarray(Wq, np.float32)
    Wk = np.asarray(Wk, np.float32)
    Wv = np.asarray(Wv, np.float32)
    Wo = np.asarray(Wo, np.float32)
    in_maps = []
    for c in range(8):
        b, hb = divmod(c, 4)
        s = slice(hb * HD, (hb + 1) * HD)
        wq_s = Wq[s].T  # (768, 192)
        wk_s = Wk[s].T
        # m0 = [Q0|Q1], m1 = [K0|K1], m2 = [Q2|K2]; each m-block stored
        # partition-major [128p, 6c, 128m] so every partition's DMA payload
        # is one contiguous 1536B run (128 descriptors, not 768).
        blocks = [np.concatenate([wq_s[:, 0:128], wk_s[:, 0:128]], axis=1),
                  np.concatenate([wq_s[:, 128:HD], wk_s[:, 128:HD]], axis=1)]
        wqk = np.stack([
            blocks[0][:, 0:128], blocks[0][:, 128:256], blocks[1],
        ]).reshape(3, KC, 128, 128).transpose(0, 2, 1, 3)  # (3, p, c, m)
        wv_pm = Wv[s].T.reshape(KC, 128, 192).transpose(1, 0, 2)  # (p, c, m)
        in_maps.append({
            "xt": np.ascontiguousarray(x[b].T).astype(bf16),
            "wqk": np.ascontiguousarray(wqk).astype(bf16),
            "wv": np.ascontiguousarray(wv_pm).astype(bf16),
            "wo": np.ascontiguousarray(Wo[:, s].T).astype(bf16),
        })
    return in_maps


def _gather(results, bo):
    out = np.zeros((B, N, C), np.float32)
    for c in range(8):
        out[c // 4] += results[c]["yt"].astype(np.float32).T
    out += np.asarray(bo, np.float32)[None, None, :]
    return out


def kernel(x, Wq, Wk, Wv, Wo, bo):
    from concourse.bass_utils import run_bass_kernel_spmd
    nc = _build_nc()
    in_maps = _make_in_maps(x, Wq, Wk, Wv, Wo)
    try:
        res = run_bass_kernel_spmd(nc, in_maps, core_ids=list(range(8)))
    except ModuleNotFoundError:
        # BASS_TRACE set but this axon deployment lacks the NTFF hook module
        import os
        os.environ["BASS_NEVER_TRACE"] = "1"
        res = run_bass_kernel_spmd(nc, in_maps, core_ids=list(range(8)))
    return _gather(res.results, bo)
